# revision 34
# baseline (speedup 1.0000x reference)
"""DecodeDetections kernel for trn2 (8 NeuronCores, SPMD data-parallel over batch).

Reference semantics (see problem):
  - decode box coords from y_pred[..., 81:93], confidences are cols 1..80
  - top-200 box indices selected from batch item 0's per-box max confidence
  - output [32, 200, 7] = (thresh_met, argmax_class, max_conf, xmin, ymin, xmax, ymax)
    gathered at those 200 indices for every batch item, ordered by descending
    batch-0 max-conf.

Strategy: each core gets 4 batch items (full rows) + a replica of batch-0's
confidence block. On-device: stream batch-0 conf -> per-box class max ->
PE-transpose into box-major [16, 4096] layout -> single gpsimd topk (k=256)
-> indirect-DMA gather of the selected 200 rows for the core's 4 batch items
-> decode only those 800 rows -> [4, 200, 7] out. Host concatenates.
"""

import numpy as np

import concourse.bass as bass
import concourse.bacc as bacc
import concourse.bass_isa as bass_isa
import concourse.mybir as mybir
import concourse.tile as tile
from concourse.masks import make_identity


def _gpsimd_topk(nc, out_ap, in_ap, tokens, vocab_size, k):
    # nc.gpsimd.topk minus the isinstance(SBTensorHandle) assert, which
    # rejects Tile-pool symbolic handles.
    eng = nc.gpsimd
    _in = eng.lower_ap(in_ap, for_isa=True)
    _out = eng.lower_ap(out_ap, for_isa=True)
    return eng.add_instruction(
        bass_isa.InstTopk(
            name=f"I-{nc.next_id()}",
            ins=[_in],
            outs=[_out],
            _tokens=tokens,
            _n=vocab_size,
            _k=k,
        )
    )

F32 = mybir.dt.float32
U32 = mybir.dt.uint32
I32 = mybir.dt.int32

N = 24564          # boxes
ROW = 93           # channels per box
NCONF = 80         # class confidences (cols 1..80)
B = 32             # total batch
NCORES = 8
BPC = B // NCORES  # batch items per core
TOPK = 200
K256 = 256
NEG = -1.0e30

CHUNK = 16                      # row-chunks of 128 per conf DMA tile
ROWS_PER_TILE = 128 * CHUNK     # 2048
NFULL = N // ROWS_PER_TILE      # 11 full tiles
TAILROWS = N - NFULL * ROWS_PER_TILE            # 2036
TAILC = TAILROWS // 128                          # 15 full c-chunks
TAILP = TAILROWS - TAILC * 128                   # 116 rows in last chunk
FTOT = 192                       # mc free dim: 24576 / 128
VOCAB = 51200                    # topk padded size (_n is u16; needs >50000, %128)
VPL = VOCAB // 16                # 3200 = 25*128 per partition


def build_nc(debug: bool = False, stage: int = 5):
    nc = _build_raw(debug, stage)
    nc.finalize()
    return nc


def _build_raw(debug: bool = False, stage: int = 5):
    nc = bacc.Bacc("TRN2", target_bir_lowering=False, debug=False)

    conf0 = nc.dram_tensor("conf0", [N, NCONF], F32, kind="ExternalInput")
    cst = nc.dram_tensor("cst", [128, 128 + NCONF], F32, kind="ExternalInput")
    yp = nc.dram_tensor("yp", [N, BPC * ROW], F32, kind="ExternalInput")  # box-major
    out = nc.dram_tensor("out", [BPC, TOPK, 7], F32, kind="ExternalOutput")
    idxb = nc.dram_tensor("idxb", [K256], U32)   # bounce: topk indices
    idxb2 = nc.dram_tensor("idxb2", [K256], U32)  # bounce: rank-ordered indices
    dbg = {}
    if debug:
        dbg["mc"] = nc.dram_tensor("dbg_mc", [128, FTOT], F32, kind="ExternalOutput")
        dbg["tko"] = nc.dram_tensor("dbg_tko", [16, 32], U32, kind="ExternalOutput")
        dbg["offs"] = nc.dram_tensor("dbg_offs", [128, 8], U32, kind="ExternalOutput")
        dbg["rank"] = nc.dram_tensor("dbg_rank", [128, 8], F32, kind="ExternalOutput")
        dbg["g"] = nc.dram_tensor("dbg_g", [128, 8, ROW], F32, kind="ExternalOutput")

    with tile.TileContext(nc) as tc:
        with (
            tc.tile_pool(name="conf", bufs=3) as conf_pool,
            tc.tile_pool(name="persist", bufs=1) as persist,
            tc.tile_pool(name="psum", bufs=2, space="PSUM") as psum_pool,
            tc.tile_pool(name="small", bufs=1) as small,
        ):
            # ---------------- persistent tiles ----------------
            mc = persist.tile([128, FTOT], F32)          # per-box class max
            topk_in = persist.tile([16, VPL], F32)       # box-major, padded
            identity = persist.tile([128, 128], F32)
            iota_f = persist.tile([128, NCONF], F32)

            # constants shipped from host (identity | iota80): using gpsimd
            # iota/affine_select would pin the standard library and delay the
            # ~30us topk-library reload to after the conf scan.
            nc.sync.dma_start(out=identity[:, :], in_=cst[:, 0:128])
            nc.sync.dma_start(out=iota_f[:, :], in_=cst[:, 128:128 + NCONF])
            # partitions 6..15 of topk_in are pure padding; 0..5 are fully
            # overwritten by the reorg DMAs below (engines need an aligned
            # start partition, so memset the whole tile)
            nc.vector.memset(topk_in[:, :], NEG)

            # ---------------- phase 1: conf scan ----------------
            for j in range(NFULL + 1):
                r0 = j * ROWS_PER_TILE
                ct = conf_pool.tile([128, CHUNK, NCONF], F32, tag="ct")
                if j < NFULL:
                    nc.sync.dma_start(
                        out=ct[:, :, :],
                        in_=conf0[r0:r0 + ROWS_PER_TILE, :].rearrange(
                            "(c p) k -> p c k", p=128),
                    )
                else:
                    # engines need aligned start partitions: memset 96..128
                    # first, the tail DMA then overwrites the valid 96..115
                    nc.vector.memset(ct[96:128, TAILC:TAILC + 1, :], NEG)
                    nc.sync.dma_start(
                        out=ct[:, 0:TAILC, :],
                        in_=conf0[r0:r0 + TAILC * 128, :].rearrange(
                            "(c p) k -> p c k", p=128),
                    )
                    nc.sync.dma_start(
                        out=ct[0:TAILP, TAILC:TAILC + 1, :],
                        in_=conf0[r0 + TAILC * 128:N, :].rearrange(
                            "(c p) k -> p c k", p=TAILP),
                    )
                nc.vector.reduce_max(
                    out=mc[:, j * CHUNK:(j + 1) * CHUNK],
                    in_=ct[:, :, :],
                    axis=mybir.AxisListType.X,
                )

            if debug:
                nc.sync.dma_start(out=dbg["mc"][:, :], in_=mc[:, :])
            if stage <= 1:
                return nc

            # ---------------- phase 2: transpose to box-major ----------------
            t1p = psum_pool.tile([128, 128], F32)
            t2p = psum_pool.tile([64, 128], F32)
            nc.tensor.transpose(t1p[:, :], mc[:, 0:128], identity[:, :])
            nc.tensor.transpose(t2p[:, :], mc[:, 128:192], identity[:, :])
            t1s = small.tile([128, 128], F32)
            t2s = small.tile([64, 128], F32)
            nc.vector.tensor_copy(t1s[:, :], t1p[:, :])
            nc.vector.tensor_copy(t2s[:, :], t2p[:, :])
            # t1s[f, p] = box(128f + p), f in [0,128)
            # t2s[f, p] = box(16384 + 128f + p), f in [0,64)
            # topk_in partition P covers boxes [3200P, 3200P+3200); the
            # T1/T2 seam (box 16384) is mid-partition-5, hence 5 DMAs.
            nc.sync.dma_start(
                out=topk_in[0:5, :].rearrange("P (a p) -> P a p", p=128),
                in_=t1s[0:125, :])
            nc.sync.dma_start(
                out=topk_in[5:6, 0:384].rearrange("P (a p) -> P a p", p=128),
                in_=t1s[125:128, :])
            nc.sync.dma_start(
                out=topk_in[5:6, 384:3200].rearrange("P (a p) -> P a p", p=128),
                in_=t2s[0:22, :])
            nc.sync.dma_start(
                out=topk_in[6:7, :].rearrange("P (a p) -> P a p", p=128),
                in_=t2s[22:47, :])
            nc.sync.dma_start(
                out=topk_in[7:8, 0:2176].rearrange("P (a p) -> P a p", p=128),
                in_=t2s[47:64, :])

            # ---------------- phase 3: topk ----------------
            tko = small.tile([16, 32], U32)
            _gpsimd_topk(nc, tko[:, :], topk_in[:, :], tokens=1,
                         vocab_size=VOCAB, k=K256)
            if debug:
                nc.sync.dma_start(out=dbg["tko"][:, :], in_=tko[:, :])
            if stage <= 2:
                return nc

            # bounce indices (topk's own order, ties arbitrary) to DRAM
            nc.sync.dma_start(out=idxb[:], in_=tko[:, 16:32])

            # ---- exact re-rank of the 256 candidates ----
            # The input has many exact-tie confidence values; jax.lax.top_k
            # orders ties by ascending box index, and the topk ucode's tie
            # order is unspecified. Recompute each candidate's exact rank:
            #   rank_c = #{j: v_j > v_c} + #{j: v_j == v_c and idx_j < idx_c}
            # candidate slot s = p + 128h  ->  (partition p, half h)
            # NOTE hw indirect DMA: ONE offset per partition, and keep all
            # indirect-call APs as clean contiguous 2-D tiles.
            icol = [small.tile([128, 1], U32, tag=f"icol{h}", name=f"icol{h}")
                    for h in range(2)]
            for h in range(2):
                nc.sync.dma_start(
                    out=icol[h][:, :],
                    in_=bass.AP(idxb[:].tensor, 128 * h, [[1, 128], [1, 1]]))
            # candidate values: gather their conf rows, reduce (bit-identical
            # to the mc values by construction)
            cgs = [small.tile([128, NCONF], F32, tag=f"cg{h}", name=f"cg{h}")
                   for h in range(2)]
            vcol = small.tile([128, 2], F32)
            idxf = small.tile([128, 2], F32)
            for h in range(2):
                nc.gpsimd.indirect_dma_start(
                    out=cgs[h][:, :], out_offset=None, in_=conf0[:, :],
                    in_offset=bass.IndirectOffsetOnAxis(ap=icol[h][:, :], axis=0))
                nc.vector.reduce_max(out=vcol[:, h:h + 1], in_=cgs[h][:, :],
                                     axis=mybir.AxisListType.X)
                nc.vector.tensor_copy(idxf[:, h:h + 1], icol[h][:, :])  # ->f32

            # broadcast all 256 (value, index) along free dim of 128 partitions
            # via DRAM bounce + step-0 replicating DMA (bit-exact; a PE
            # ones-matmul broadcast is NOT bit-exact on hw fp32)
            vb = nc.dram_tensor("vb", [K256], F32)
            nc.sync.dma_start(
                out=bass.AP(vb[:].tensor, 0, [[1, 128], [128, 2]]),
                in_=vcol[:, :])
            vrow_ps = small.tile([128, K256], F32)
            nc.sync.dma_start(
                out=vrow_ps[:, :],
                in_=bass.AP(vb[:].tensor, 0, [[0, 128], [1, K256]]))
            irow_u = small.tile([128, K256], U32)
            nc.sync.dma_start(
                out=irow_u[:, :],
                in_=bass.AP(idxb[:].tensor, 0, [[0, 128], [1, K256]]))
            irow_ps = small.tile([128, K256], F32)
            nc.vector.tensor_copy(irow_ps[:, :], irow_u[:, :])

            frank = small.tile([128, 2], F32)
            if debug:
                dbgrank = small.tile([128, 8], F32)
                nc.vector.tensor_copy(dbgrank[:, 0:2], vcol[:, :])
                nc.vector.tensor_copy(dbgrank[:, 2:4], idxf[:, :])
            for h in range(2):
                j1 = small.tile([128, K256], F32, tag=f"j1_{h}")
                eqm = small.tile([128, K256], F32, tag=f"eq_{h}")
                j2 = small.tile([128, K256], F32, tag=f"j2_{h}")
                r1 = small.tile([128, 1], F32, tag=f"r1_{h}")
                r2 = small.tile([128, 1], F32, tag=f"r2_{h}")
                nc.vector.tensor_scalar(
                    out=j1[:, :], in0=vrow_ps[:, :], scalar1=vcol[:, h:h + 1],
                    scalar2=None, op0=mybir.AluOpType.is_gt,
                    op1=mybir.AluOpType.add, accum_out=r1[:, :])
                nc.vector.tensor_scalar(
                    out=eqm[:, :], in0=vrow_ps[:, :], scalar1=vcol[:, h:h + 1],
                    scalar2=None, op0=mybir.AluOpType.is_equal)
                nc.vector.scalar_tensor_tensor(
                    out=j2[:, :], in0=irow_ps[:, :], scalar=idxf[:, h:h + 1],
                    in1=eqm[:, :], op0=mybir.AluOpType.is_lt,
                    op1=mybir.AluOpType.mult)
                nc.vector.reduce_sum(out=r2[:, :], in_=j2[:, :],
                                     axis=mybir.AxisListType.X)
                nc.vector.tensor_tensor(out=frank[:, h:h + 1], in0=r1[:, :],
                                        in1=r2[:, :], op=mybir.AluOpType.add)
                if debug:
                    nc.vector.tensor_copy(dbgrank[:, 4 + h:5 + h], r1[:, :])
                    nc.vector.tensor_copy(dbgrank[:, 6 + h:7 + h], r2[:, :])

            # scatter candidate indices to DRAM position = final rank
            for h in range(2):
                franku = small.tile([128, 1], U32, tag=f"fru{h}")
                nc.vector.tensor_copy(franku[:, :], frank[:, h:h + 1])
                nc.gpsimd.indirect_dma_start(
                    out=idxb2[:].rearrange("(a b) -> a b", b=1),
                    out_offset=bass.IndirectOffsetOnAxis(ap=franku[:, :], axis=0),
                    in_=icol[h][:, :], in_offset=None)

            # bo[h][p] = box index with final rank d = 128*half + p
            bo = [small.tile([128, 1], U32, tag=f"bo{h}", name=f"bo{h}")
                  for h in range(2)]
            for h in range(2):
                nc.sync.dma_start(
                    out=bo[h][:, :],
                    in_=bass.AP(idxb2[:].tensor, 128 * h, [[1, 128], [1, 1]]))
            if debug:
                offs_mega = small.tile([128, 8], U32)
                nc.vector.memset(offs_mega[:, :], 0)
                for h in range(2):
                    nc.vector.tensor_copy(offs_mega[:, h:h + 1], bo[h][:, :])
                nc.sync.dma_start(out=dbg["offs"][:, :], in_=offs_mega[:, :])
                nc.sync.dma_start(out=dbg["rank"][:, :], in_=dbgrank[:, :])
            if stage <= 3:
                return nc

            # ---------------- phase 4: gather ----------------
            # yp is box-major [N, 4*93]: one index fetches all 4 batch rows.
            # g column i = 4*half + bb.
            g = persist.tile([128, 8, ROW], F32)
            for h in range(2):
                gh = small.tile([128, BPC * ROW], F32, tag=f"gh{h}", name=f"gh{h}")
                nc.gpsimd.indirect_dma_start(
                    out=gh[:, :],
                    out_offset=None,
                    in_=yp[:, :],
                    in_offset=bass.IndirectOffsetOnAxis(ap=bo[h][:, :], axis=0),
                )
                nc.vector.tensor_copy(g[:, 4 * h:4 * h + 4, :],
                                      gh[:, :].rearrange("p (b r) -> p b r", r=ROW))
            if debug:
                nc.sync.dma_start(out=dbg["g"][:, :, :], in_=g[:, :, :])
            if stage <= 4:
                return nc

            # ---------------- phase 5: decode ----------------
            out7 = persist.tile([128, 8, 7], F32)
            conf = g[:, :, 1:1 + NCONF]                    # [128, 8, 80]
            mxc = small.tile([128, 8], F32)
            nc.vector.reduce_max(out=mxc[:, :], in_=conf, axis=mybir.AxisListType.X)

            # argmax via (iota - 256*eq) reduce_min
            eq = small.tile([128, 8, NCONF], F32)
            mxc_b = bass.AP(mxc[:, :].tensor, mxc[:, :].offset,
                            [list(mxc[:, :].ap[0]), list(mxc[:, :].ap[1]), [0, NCONF]])
            nc.vector.tensor_tensor(out=eq[:, :, :], in0=conf, in1=mxc_b,
                                    op=mybir.AluOpType.is_equal)
            iota_b = bass.AP(iota_f[:, :].tensor, iota_f[:, :].offset,
                             [list(iota_f[:, :].ap[0]), [0, 8], [1, NCONF]])
            cand = small.tile([128, 8, NCONF], F32)
            nc.vector.scalar_tensor_tensor(
                out=cand[:, :, :], in0=eq[:, :, :], scalar=-256.0, in1=iota_b,
                op0=mybir.AluOpType.mult, op1=mybir.AluOpType.add)
            amx = small.tile([128, 8], F32)
            nc.vector.tensor_reduce(out=amx[:, :], in_=cand[:, :, :],
                                    axis=mybir.AxisListType.X,
                                    op=mybir.AluOpType.min)
            nc.vector.tensor_scalar(out=out7[:, :, 1], in0=amx[:, :], scalar1=256.0,
                                    scalar2=None, op0=mybir.AluOpType.add)
            nc.vector.tensor_scalar(out=out7[:, :, 0], in0=mxc[:, :], scalar1=0.5,
                                    scalar2=None, op0=mybir.AluOpType.is_gt)
            nc.vector.tensor_copy(out7[:, :, 2], mxc[:, :])

            def col(k):
                return g[:, :, 81 + k]

            tmp = small.tile([128, 8], F32)
            cx = small.tile([128, 8], F32)
            cy = small.tile([128, 8], F32)
            w5 = small.tile([128, 8], F32)
            h5 = small.tile([128, 8], F32)

            # cx = ((c0*c8)*c6 + c4) ; cy = ((c1*c9)*c7 + c5)
            nc.vector.tensor_tensor(out=tmp[:, :], in0=col(0), in1=col(8),
                                    op=mybir.AluOpType.mult)
            nc.vector.tensor_tensor(out=tmp[:, :], in0=tmp[:, :], in1=col(6),
                                    op=mybir.AluOpType.mult)
            nc.vector.tensor_tensor(out=cx[:, :], in0=tmp[:, :], in1=col(4),
                                    op=mybir.AluOpType.add)
            nc.vector.tensor_tensor(out=tmp[:, :], in0=col(1), in1=col(9),
                                    op=mybir.AluOpType.mult)
            nc.vector.tensor_tensor(out=tmp[:, :], in0=tmp[:, :], in1=col(7),
                                    op=mybir.AluOpType.mult)
            nc.vector.tensor_tensor(out=cy[:, :], in0=tmp[:, :], in1=col(5),
                                    op=mybir.AluOpType.add)
            # w = exp(c2*c10)*c6 ; h = exp(c3*c11)*c7   (then * 512)
            # Precise f32 exp on DVE (ACT's Exp LUT is only ~2e-4 accurate):
            # k = round(x/ln2) via the magic-constant trick, 3-term
            # Cody-Waite reduction, degree-7 Taylor Horner, exact 2^k by
            # integer-constructing the f32 bit pattern and bitcasting.
            INV_LN2 = 1.4426950408889634
            MAGIC = 12582912.0          # 1.5 * 2^23: round-to-nearest
            CW1, CW2, CW3 = 0.693359375, -2.1219444e-4, 1.6465718e-12
            FACT = [1.0, 1.0, 0.5, 1.0 / 6, 1.0 / 24, 1.0 / 120, 1.0 / 720,
                    1.0 / 5040]
            xe = small.tile([128, 16], F32)
            nc.vector.tensor_tensor(out=xe[:, 0:8], in0=col(2), in1=col(10),
                                    op=mybir.AluOpType.mult)
            nc.vector.tensor_tensor(out=xe[:, 8:16], in0=col(3), in1=col(11),
                                    op=mybir.AluOpType.mult)
            kf = small.tile([128, 16], F32)
            nc.vector.tensor_scalar(out=kf[:, :], in0=xe[:, :], scalar1=INV_LN2,
                                    scalar2=None, op0=mybir.AluOpType.mult)
            nc.vector.tensor_scalar(out=kf[:, :], in0=kf[:, :], scalar1=MAGIC,
                                    scalar2=MAGIC, op0=mybir.AluOpType.add,
                                    op1=mybir.AluOpType.subtract)
            rr = small.tile([128, 16], F32)
            nc.vector.scalar_tensor_tensor(
                out=rr[:, :], in0=kf[:, :], scalar=-CW1, in1=xe[:, :],
                op0=mybir.AluOpType.mult, op1=mybir.AluOpType.add)
            nc.vector.scalar_tensor_tensor(
                out=rr[:, :], in0=kf[:, :], scalar=-CW2, in1=rr[:, :],
                op0=mybir.AluOpType.mult, op1=mybir.AluOpType.add)
            nc.vector.scalar_tensor_tensor(
                out=rr[:, :], in0=kf[:, :], scalar=-CW3, in1=rr[:, :],
                op0=mybir.AluOpType.mult, op1=mybir.AluOpType.add)
            pp = small.tile([128, 16], F32)
            pq = small.tile([128, 16], F32)
            nc.vector.memset(pp[:, :], FACT[7])
            for kdeg in range(6, -1, -1):
                nc.vector.tensor_tensor(out=pq[:, :], in0=pp[:, :], in1=rr[:, :],
                                        op=mybir.AluOpType.mult)
                nc.vector.tensor_scalar(out=pp[:, :], in0=pq[:, :],
                                        scalar1=FACT[kdeg], scalar2=None,
                                        op0=mybir.AluOpType.add)
            # 2^k: bits = (k+127) * 2^23, exact in f32; value-cast to u32
            # and bitcast back to f32
            bitsf = small.tile([128, 16], F32)
            nc.vector.tensor_scalar(out=bitsf[:, :], in0=kf[:, :], scalar1=127.0,
                                    scalar2=8388608.0, op0=mybir.AluOpType.add,
                                    op1=mybir.AluOpType.mult)
            bitsu = small.tile([128, 16], U32)
            nc.vector.tensor_copy(bitsu[:, :], bitsf[:, :])
            exv = small.tile([128, 16], F32)
            nc.vector.tensor_tensor(out=exv[:, :], in0=pp[:, :],
                                    in1=bitsu[:, :].bitcast(F32),
                                    op=mybir.AluOpType.mult)
            nc.vector.tensor_tensor(out=w5[:, :], in0=exv[:, 0:8], in1=col(6),
                                    op=mybir.AluOpType.mult)
            nc.vector.tensor_tensor(out=h5[:, :], in0=exv[:, 8:16], in1=col(7),
                                    op=mybir.AluOpType.mult)
            # scale by 512 (exact)
            nc.vector.tensor_scalar_mul(cx[:, :], cx[:, :], 512.0)
            nc.vector.tensor_scalar_mul(cy[:, :], cy[:, :], 512.0)
            nc.vector.tensor_scalar_mul(w5[:, :], w5[:, :], 512.0)
            nc.vector.tensor_scalar_mul(h5[:, :], h5[:, :], 512.0)
            # corners
            nc.vector.scalar_tensor_tensor(out=out7[:, :, 3], in0=w5[:, :],
                                           scalar=-0.5, in1=cx[:, :],
                                           op0=mybir.AluOpType.mult,
                                           op1=mybir.AluOpType.add)
            nc.vector.scalar_tensor_tensor(out=out7[:, :, 4], in0=h5[:, :],
                                           scalar=-0.5, in1=cy[:, :],
                                           op0=mybir.AluOpType.mult,
                                           op1=mybir.AluOpType.add)
            nc.vector.scalar_tensor_tensor(out=out7[:, :, 5], in0=w5[:, :],
                                           scalar=0.5, in1=cx[:, :],
                                           op0=mybir.AluOpType.mult,
                                           op1=mybir.AluOpType.add)
            nc.vector.scalar_tensor_tensor(out=out7[:, :, 6], in0=h5[:, :],
                                           scalar=0.5, in1=cy[:, :],
                                           op0=mybir.AluOpType.mult,
                                           op1=mybir.AluOpType.add)

            # ---------------- phase 6: write out ----------------
            # out[bb, d, :] with d = 128*half + p lives at out7[p, 2bb+half, :]
            out_ap0 = bass.AP(out[:, :, :].tensor, 0,
                              [[7, 128], [TOPK * 7, BPC], [1, 7]])
            nc.sync.dma_start(out=out_ap0, in_=out7[:, 0:4, :])
            out_ap1 = bass.AP(out[:, :, :].tensor, 128 * 7,
                              [[7, 72], [TOPK * 7, BPC], [1, 7]])
            nc.sync.dma_start(out=out_ap1, in_=out7[0:72, 4:8, :])

    return nc


_cached_nc = None

# test-harness knobs (ignored in normal use)
TRACE = False
LAST_RESULTS = None


def kernel(y_pred: np.ndarray) -> np.ndarray:
    from concourse.bass_utils import run_bass_kernel_spmd

    global _cached_nc, LAST_RESULTS
    if _cached_nc is None:
        _cached_nc = build_nc(debug=False)
    nc = _cached_nc

    y_pred = np.asarray(y_pred, dtype=np.float32)
    conf0 = np.ascontiguousarray(y_pred[0, :, 1:1 + NCONF])
    cst = np.zeros((128, 128 + NCONF), np.float32)
    cst[:, 0:128] = np.eye(128, dtype=np.float32)
    cst[:, 128:] = np.arange(NCONF, dtype=np.float32)[None, :]
    in_maps = []
    for c in range(NCORES):
        shard = np.ascontiguousarray(
            y_pred[c * BPC:(c + 1) * BPC].transpose(1, 0, 2).reshape(N, BPC * ROW))
        in_maps.append({"conf0": conf0, "yp": shard, "cst": cst})

    res = run_bass_kernel_spmd(nc, in_maps, core_ids=list(range(NCORES)),
                               trace=TRACE)
    LAST_RESULTS = res
    out = np.concatenate([res.results[c]["out"] for c in range(NCORES)], axis=0)
    return out


# revision 35
# speedup vs baseline: 1.0903x; 1.0903x over previous
"""DecodeDetections kernel for trn2 (8 NeuronCores, SPMD data-parallel over batch).

Reference semantics (see problem):
  - decode box coords from y_pred[..., 81:93], confidences are cols 1..80
  - top-200 box indices selected from batch item 0's per-box max confidence
  - output [32, 200, 7] = (thresh_met, argmax_class, max_conf, xmin, ymin, xmax, ymax)
    gathered at those 200 indices for every batch item, ordered by descending
    batch-0 max-conf.

Strategy: each core gets 4 batch items (full rows) + a replica of batch-0's
confidence block. On-device: stream batch-0 conf -> per-box class max ->
PE-transpose into box-major [16, 4096] layout -> single gpsimd topk (k=256)
-> indirect-DMA gather of the selected 200 rows for the core's 4 batch items
-> decode only those 800 rows -> [4, 200, 7] out. Host concatenates.
"""

import numpy as np

import concourse.bass as bass
import concourse.bacc as bacc
import concourse.bass_isa as bass_isa
import concourse.mybir as mybir
import concourse.tile as tile
from concourse import library_config


def _gpsimd_topk(nc, out_ap, in_ap, tokens, vocab_size, k):
    # nc.gpsimd.topk minus the isinstance(SBTensorHandle) assert, which
    # rejects Tile-pool symbolic handles.
    eng = nc.gpsimd
    _in = eng.lower_ap(in_ap, for_isa=True)
    _out = eng.lower_ap(out_ap, for_isa=True)
    return eng.add_instruction(
        bass_isa.InstTopk(
            name=f"I-{nc.next_id()}",
            ins=[_in],
            outs=[_out],
            _tokens=tokens,
            _n=vocab_size,
            _k=k,
        )
    )

F32 = mybir.dt.float32
U32 = mybir.dt.uint32
I32 = mybir.dt.int32

N = 24564          # boxes
ROW = 93           # channels per box
NCONF = 80         # class confidences (cols 1..80)
B = 32             # total batch
NCORES = 8
BPC = B // NCORES  # batch items per core
TOPK = 200
K256 = 256
NEG = -1.0e30

CHUNK = 16                      # row-chunks of 128 per conf DMA tile
ROWS_PER_TILE = 128 * CHUNK     # 2048
NFULL = N // ROWS_PER_TILE      # 11 full tiles
TAILROWS = N - NFULL * ROWS_PER_TILE            # 2036
TAILC = TAILROWS // 128                          # 15 full c-chunks
TAILP = TAILROWS - TAILC * 128                   # 116 rows in last chunk
FTOT = 192                       # mc free dim: 24576 / 128
VOCAB = 51200                    # topk padded size (_n is u16; needs >50000, %128)
VPL = VOCAB // 16                # 3200 = 25*128 per partition


def build_nc(debug: bool = False, stage: int = 5):
    nc = _build_raw(debug, stage)
    nc.finalize()
    return nc


def _build_raw(debug: bool = False, stage: int = 5):
    nc = bacc.Bacc("TRN2", target_bir_lowering=False, debug=False)

    conf0 = nc.dram_tensor("conf0", [N, NCONF], F32, kind="ExternalInput")
    cst = nc.dram_tensor("cst", [128, 128 + NCONF], F32, kind="ExternalInput")
    yp = nc.dram_tensor("yp", [N, BPC * ROW], F32, kind="ExternalInput")  # box-major
    out = nc.dram_tensor("out", [BPC, TOPK, 7], F32, kind="ExternalOutput")
    idxb = nc.dram_tensor("idxb", [K256], U32)   # bounce: topk indices
    idxb2 = nc.dram_tensor("idxb2", [K256], U32)  # bounce: rank-ordered indices
    dbg = {}
    if debug:
        dbg["mc"] = nc.dram_tensor("dbg_mc", [128, FTOT], F32, kind="ExternalOutput")
        dbg["tko"] = nc.dram_tensor("dbg_tko", [16, 32], U32, kind="ExternalOutput")
        dbg["offs"] = nc.dram_tensor("dbg_offs", [128, 8], U32, kind="ExternalOutput")
        dbg["rank"] = nc.dram_tensor("dbg_rank", [128, 8], F32, kind="ExternalOutput")
        dbg["g"] = nc.dram_tensor("dbg_g", [128, 8, ROW], F32, kind="ExternalOutput")

    with tile.TileContext(nc) as tc:
        with (
            tc.tile_pool(name="conf", bufs=3) as conf_pool,
            tc.tile_pool(name="persist", bufs=1) as persist,
            tc.tile_pool(name="psum", bufs=2, space="PSUM") as psum_pool,
            tc.tile_pool(name="small", bufs=1) as small,
        ):
            # ---------------- persistent tiles ----------------
            mc = persist.tile([128, FTOT], F32)          # per-box class max
            topk_in = persist.tile([16, VPL], F32)       # box-major, padded
            identity = persist.tile([128, 128], F32)
            iota_f = persist.tile([128, NCONF], F32)

            # load the topk gpsimd library immediately: the ~30us ucode IRAM
            # fetch then overlaps the conf scan instead of serializing right
            # before the topk instruction.
            nc.gpsimd.load_library(library_config.topk)
            # constants shipped from host (identity | iota80): using gpsimd
            # iota/affine_select would pin the standard library and delay the
            # ~30us topk-library reload to after the conf scan.
            nc.sync.dma_start(out=identity[:, :], in_=cst[:, 0:128])
            nc.sync.dma_start(out=iota_f[:, :], in_=cst[:, 128:128 + NCONF])
            # partitions 6..15 of topk_in are pure padding; 0..5 are fully
            # overwritten by the reorg DMAs below (engines need an aligned
            # start partition, so memset the whole tile)
            nc.vector.memset(topk_in[:, :], NEG)

            # ---------------- phase 1: conf scan ----------------
            for j in range(NFULL + 1):
                r0 = j * ROWS_PER_TILE
                ct = conf_pool.tile([128, CHUNK, NCONF], F32, tag="ct")
                if j < NFULL:
                    nc.sync.dma_start(
                        out=ct[:, :, :],
                        in_=conf0[r0:r0 + ROWS_PER_TILE, :].rearrange(
                            "(c p) k -> p c k", p=128),
                    )
                else:
                    # engines need aligned start partitions: memset 96..128
                    # first, the tail DMA then overwrites the valid 96..115
                    nc.vector.memset(ct[96:128, TAILC:TAILC + 1, :], NEG)
                    nc.sync.dma_start(
                        out=ct[:, 0:TAILC, :],
                        in_=conf0[r0:r0 + TAILC * 128, :].rearrange(
                            "(c p) k -> p c k", p=128),
                    )
                    nc.sync.dma_start(
                        out=ct[0:TAILP, TAILC:TAILC + 1, :],
                        in_=conf0[r0 + TAILC * 128:N, :].rearrange(
                            "(c p) k -> p c k", p=TAILP),
                    )
                nc.vector.reduce_max(
                    out=mc[:, j * CHUNK:(j + 1) * CHUNK],
                    in_=ct[:, :, :],
                    axis=mybir.AxisListType.X,
                )

            if debug:
                nc.sync.dma_start(out=dbg["mc"][:, :], in_=mc[:, :])
            if stage <= 1:
                return nc

            # ---------------- phase 2: transpose to box-major ----------------
            t1p = psum_pool.tile([128, 128], F32)
            t2p = psum_pool.tile([64, 128], F32)
            nc.tensor.transpose(t1p[:, :], mc[:, 0:128], identity[:, :])
            nc.tensor.transpose(t2p[:, :], mc[:, 128:192], identity[:, :])
            t1s = small.tile([128, 128], F32)
            t2s = small.tile([64, 128], F32)
            nc.vector.tensor_copy(t1s[:, :], t1p[:, :])
            nc.vector.tensor_copy(t2s[:, :], t2p[:, :])
            # t1s[f, p] = box(128f + p), f in [0,128)
            # t2s[f, p] = box(16384 + 128f + p), f in [0,64)
            # topk_in partition P covers boxes [3200P, 3200P+3200); the
            # T1/T2 seam (box 16384) is mid-partition-5, hence 5 DMAs.
            nc.sync.dma_start(
                out=topk_in[0:5, :].rearrange("P (a p) -> P a p", p=128),
                in_=t1s[0:125, :])
            nc.sync.dma_start(
                out=topk_in[5:6, 0:384].rearrange("P (a p) -> P a p", p=128),
                in_=t1s[125:128, :])
            nc.sync.dma_start(
                out=topk_in[5:6, 384:3200].rearrange("P (a p) -> P a p", p=128),
                in_=t2s[0:22, :])
            nc.sync.dma_start(
                out=topk_in[6:7, :].rearrange("P (a p) -> P a p", p=128),
                in_=t2s[22:47, :])
            nc.sync.dma_start(
                out=topk_in[7:8, 0:2176].rearrange("P (a p) -> P a p", p=128),
                in_=t2s[47:64, :])

            # ---------------- phase 3: topk ----------------
            tko = small.tile([16, 32], U32)
            _gpsimd_topk(nc, tko[:, :], topk_in[:, :], tokens=1,
                         vocab_size=VOCAB, k=K256)
            if debug:
                nc.sync.dma_start(out=dbg["tko"][:, :], in_=tko[:, :])
            if stage <= 2:
                return nc

            # bounce indices (topk's own order, ties arbitrary) to DRAM
            nc.sync.dma_start(out=idxb[:], in_=tko[:, 16:32])

            # ---- exact re-rank of the 256 candidates ----
            # The input has many exact-tie confidence values; jax.lax.top_k
            # orders ties by ascending box index, and the topk ucode's tie
            # order is unspecified. Recompute each candidate's exact rank:
            #   rank_c = #{j: v_j > v_c} + #{j: v_j == v_c and idx_j < idx_c}
            # candidate slot s = p + 128h  ->  (partition p, half h)
            # NOTE hw indirect DMA: ONE offset per partition, and keep all
            # indirect-call APs as clean contiguous 2-D tiles.
            icol = [small.tile([128, 1], U32, tag=f"icol{h}", name=f"icol{h}")
                    for h in range(2)]
            for h in range(2):
                nc.sync.dma_start(
                    out=icol[h][:, :],
                    in_=bass.AP(idxb[:].tensor, 128 * h, [[1, 128], [1, 1]]))
            # candidate values come straight from the topk output's value
            # half (same f32 bits as mc) -- bounce to DRAM, read back as a
            # per-partition column and a 128-way broadcast row, bitcast u32->f32
            vb0 = nc.dram_tensor("vb0", [K256], U32)
            nc.sync.dma_start(out=vb0[:], in_=tko[:, 0:16])
            vcol_u = small.tile([128, 2], U32)
            nc.sync.dma_start(
                out=vcol_u[:, :],
                in_=bass.AP(vb0[:].tensor, 0, [[1, 128], [128, 2]]))
            vcol = vcol_u[:, :].bitcast(F32)
            idxf = small.tile([128, 2], F32)
            for h in range(2):
                nc.vector.tensor_copy(idxf[:, h:h + 1], icol[h][:, :])  # ->f32
            vrow_u = small.tile([128, K256], U32)
            nc.sync.dma_start(
                out=vrow_u[:, :],
                in_=bass.AP(vb0[:].tensor, 0, [[0, 128], [1, K256]]))
            vrow = vrow_u[:, :].bitcast(F32)
            irow_u = small.tile([128, K256], U32)
            nc.sync.dma_start(
                out=irow_u[:, :],
                in_=bass.AP(idxb[:].tensor, 0, [[0, 128], [1, K256]]))
            irow_ps = small.tile([128, K256], F32)
            nc.vector.tensor_copy(irow_ps[:, :], irow_u[:, :])

            frank = small.tile([128, 2], F32)
            for h in range(2):
                j1 = small.tile([128, K256], F32, tag=f"j1_{h}")
                eqm = small.tile([128, K256], F32, tag=f"eq_{h}")
                j2 = small.tile([128, K256], F32, tag=f"j2_{h}")
                r1 = small.tile([128, 1], F32, tag=f"r1_{h}")
                r2 = small.tile([128, 1], F32, tag=f"r2_{h}")
                nc.vector.tensor_scalar(
                    out=j1[:, :], in0=vrow, scalar1=vcol[:, h:h + 1],
                    scalar2=None, op0=mybir.AluOpType.is_gt,
                    op1=mybir.AluOpType.add, accum_out=r1[:, :])
                nc.vector.tensor_scalar(
                    out=eqm[:, :], in0=vrow, scalar1=vcol[:, h:h + 1],
                    scalar2=None, op0=mybir.AluOpType.is_equal)
                nc.vector.scalar_tensor_tensor(
                    out=j2[:, :], in0=irow_ps[:, :], scalar=idxf[:, h:h + 1],
                    in1=eqm[:, :], op0=mybir.AluOpType.is_lt,
                    op1=mybir.AluOpType.mult)
                nc.vector.reduce_sum(out=r2[:, :], in_=j2[:, :],
                                     axis=mybir.AxisListType.X)
                nc.vector.tensor_tensor(out=frank[:, h:h + 1], in0=r1[:, :],
                                        in1=r2[:, :], op=mybir.AluOpType.add)

            # scatter candidate indices to DRAM position = final rank
            for h in range(2):
                franku = small.tile([128, 1], U32, tag=f"fru{h}")
                nc.vector.tensor_copy(franku[:, :], frank[:, h:h + 1])
                nc.gpsimd.indirect_dma_start(
                    out=idxb2[:].rearrange("(a b) -> a b", b=1),
                    out_offset=bass.IndirectOffsetOnAxis(ap=franku[:, :], axis=0),
                    in_=icol[h][:, :], in_offset=None)

            # bo[h][p] = box index with final rank d = 128*half + p
            bo = [small.tile([128, 1], U32, tag=f"bo{h}", name=f"bo{h}")
                  for h in range(2)]
            for h in range(2):
                nc.sync.dma_start(
                    out=bo[h][:, :],
                    in_=bass.AP(idxb2[:].tensor, 128 * h, [[1, 128], [1, 1]]))
            if debug:
                offs_mega = small.tile([128, 8], U32)
                nc.vector.memset(offs_mega[:, :], 0)
                for h in range(2):
                    nc.vector.tensor_copy(offs_mega[:, h:h + 1], bo[h][:, :])
                nc.sync.dma_start(out=dbg["offs"][:, :], in_=offs_mega[:, :])
            if stage <= 3:
                return nc

            # ---------------- phase 4: gather ----------------
            # yp is box-major [N, 4*93]: one index fetches all 4 batch rows.
            # g column i = 4*half + bb.
            g = persist.tile([128, 8, ROW], F32)
            for h in range(2):
                gh = small.tile([128, BPC * ROW], F32, tag=f"gh{h}", name=f"gh{h}")
                nc.gpsimd.indirect_dma_start(
                    out=gh[:, :],
                    out_offset=None,
                    in_=yp[:, :],
                    in_offset=bass.IndirectOffsetOnAxis(ap=bo[h][:, :], axis=0),
                )
                nc.vector.tensor_copy(g[:, 4 * h:4 * h + 4, :],
                                      gh[:, :].rearrange("p (b r) -> p b r", r=ROW))
            if debug:
                nc.sync.dma_start(out=dbg["g"][:, :, :], in_=g[:, :, :])
            if stage <= 4:
                return nc

            # ---------------- phase 5: decode ----------------
            out7 = persist.tile([128, 8, 7], F32)
            conf = g[:, :, 1:1 + NCONF]                    # [128, 8, 80]
            mxc = small.tile([128, 8], F32)
            nc.vector.reduce_max(out=mxc[:, :], in_=conf, axis=mybir.AxisListType.X)

            # argmax via (iota - 256*eq) reduce_min
            eq = small.tile([128, 8, NCONF], F32)
            mxc_b = bass.AP(mxc[:, :].tensor, mxc[:, :].offset,
                            [list(mxc[:, :].ap[0]), list(mxc[:, :].ap[1]), [0, NCONF]])
            nc.vector.tensor_tensor(out=eq[:, :, :], in0=conf, in1=mxc_b,
                                    op=mybir.AluOpType.is_equal)
            iota_b = bass.AP(iota_f[:, :].tensor, iota_f[:, :].offset,
                             [list(iota_f[:, :].ap[0]), [0, 8], [1, NCONF]])
            cand = small.tile([128, 8, NCONF], F32)
            nc.vector.scalar_tensor_tensor(
                out=cand[:, :, :], in0=eq[:, :, :], scalar=-256.0, in1=iota_b,
                op0=mybir.AluOpType.mult, op1=mybir.AluOpType.add)
            amx = small.tile([128, 8], F32)
            nc.vector.tensor_reduce(out=amx[:, :], in_=cand[:, :, :],
                                    axis=mybir.AxisListType.X,
                                    op=mybir.AluOpType.min)
            nc.vector.tensor_scalar(out=out7[:, :, 1], in0=amx[:, :], scalar1=256.0,
                                    scalar2=None, op0=mybir.AluOpType.add)
            nc.vector.tensor_scalar(out=out7[:, :, 0], in0=mxc[:, :], scalar1=0.5,
                                    scalar2=None, op0=mybir.AluOpType.is_gt)
            nc.vector.tensor_copy(out7[:, :, 2], mxc[:, :])

            def col(k):
                return g[:, :, 81 + k]

            tmp = small.tile([128, 8], F32)
            cx = small.tile([128, 8], F32)
            cy = small.tile([128, 8], F32)
            w5 = small.tile([128, 8], F32)
            h5 = small.tile([128, 8], F32)

            # cx = ((c0*c8)*c6 + c4) ; cy = ((c1*c9)*c7 + c5)
            nc.vector.tensor_tensor(out=tmp[:, :], in0=col(0), in1=col(8),
                                    op=mybir.AluOpType.mult)
            nc.vector.tensor_tensor(out=tmp[:, :], in0=tmp[:, :], in1=col(6),
                                    op=mybir.AluOpType.mult)
            nc.vector.tensor_tensor(out=cx[:, :], in0=tmp[:, :], in1=col(4),
                                    op=mybir.AluOpType.add)
            nc.vector.tensor_tensor(out=tmp[:, :], in0=col(1), in1=col(9),
                                    op=mybir.AluOpType.mult)
            nc.vector.tensor_tensor(out=tmp[:, :], in0=tmp[:, :], in1=col(7),
                                    op=mybir.AluOpType.mult)
            nc.vector.tensor_tensor(out=cy[:, :], in0=tmp[:, :], in1=col(5),
                                    op=mybir.AluOpType.add)
            # w = exp(c2*c10)*c6 ; h = exp(c3*c11)*c7   (then * 512)
            # Precise f32 exp on DVE (ACT's Exp LUT is only ~2e-4 accurate):
            # k = round(x/ln2) via the magic-constant trick, 3-term
            # Cody-Waite reduction, degree-7 Taylor Horner, exact 2^k by
            # integer-constructing the f32 bit pattern and bitcasting.
            INV_LN2 = 1.4426950408889634
            MAGIC = 12582912.0          # 1.5 * 2^23: round-to-nearest
            CW1, CW2, CW3 = 0.693359375, -2.1219444e-4, 1.6465718e-12
            FACT = [1.0, 1.0, 0.5, 1.0 / 6, 1.0 / 24, 1.0 / 120, 1.0 / 720,
                    1.0 / 5040]
            xe = small.tile([128, 16], F32)
            nc.vector.tensor_tensor(out=xe[:, 0:8], in0=col(2), in1=col(10),
                                    op=mybir.AluOpType.mult)
            nc.vector.tensor_tensor(out=xe[:, 8:16], in0=col(3), in1=col(11),
                                    op=mybir.AluOpType.mult)
            kf = small.tile([128, 16], F32)
            nc.vector.tensor_scalar(out=kf[:, :], in0=xe[:, :], scalar1=INV_LN2,
                                    scalar2=None, op0=mybir.AluOpType.mult)
            nc.vector.tensor_scalar(out=kf[:, :], in0=kf[:, :], scalar1=MAGIC,
                                    scalar2=MAGIC, op0=mybir.AluOpType.add,
                                    op1=mybir.AluOpType.subtract)
            rr = small.tile([128, 16], F32)
            nc.vector.scalar_tensor_tensor(
                out=rr[:, :], in0=kf[:, :], scalar=-CW1, in1=xe[:, :],
                op0=mybir.AluOpType.mult, op1=mybir.AluOpType.add)
            nc.vector.scalar_tensor_tensor(
                out=rr[:, :], in0=kf[:, :], scalar=-CW2, in1=rr[:, :],
                op0=mybir.AluOpType.mult, op1=mybir.AluOpType.add)
            nc.vector.scalar_tensor_tensor(
                out=rr[:, :], in0=kf[:, :], scalar=-CW3, in1=rr[:, :],
                op0=mybir.AluOpType.mult, op1=mybir.AluOpType.add)
            pp = small.tile([128, 16], F32)
            pq = small.tile([128, 16], F32)
            nc.vector.memset(pp[:, :], FACT[7])
            for kdeg in range(6, -1, -1):
                nc.vector.tensor_tensor(out=pq[:, :], in0=pp[:, :], in1=rr[:, :],
                                        op=mybir.AluOpType.mult)
                nc.vector.tensor_scalar(out=pp[:, :], in0=pq[:, :],
                                        scalar1=FACT[kdeg], scalar2=None,
                                        op0=mybir.AluOpType.add)
            # 2^k: bits = (k+127) * 2^23, exact in f32; value-cast to u32
            # and bitcast back to f32
            bitsf = small.tile([128, 16], F32)
            nc.vector.tensor_scalar(out=bitsf[:, :], in0=kf[:, :], scalar1=127.0,
                                    scalar2=8388608.0, op0=mybir.AluOpType.add,
                                    op1=mybir.AluOpType.mult)
            bitsu = small.tile([128, 16], U32)
            nc.vector.tensor_copy(bitsu[:, :], bitsf[:, :])
            exv = small.tile([128, 16], F32)
            nc.vector.tensor_tensor(out=exv[:, :], in0=pp[:, :],
                                    in1=bitsu[:, :].bitcast(F32),
                                    op=mybir.AluOpType.mult)
            nc.vector.tensor_tensor(out=w5[:, :], in0=exv[:, 0:8], in1=col(6),
                                    op=mybir.AluOpType.mult)
            nc.vector.tensor_tensor(out=h5[:, :], in0=exv[:, 8:16], in1=col(7),
                                    op=mybir.AluOpType.mult)
            # scale by 512 (exact)
            nc.vector.tensor_scalar_mul(cx[:, :], cx[:, :], 512.0)
            nc.vector.tensor_scalar_mul(cy[:, :], cy[:, :], 512.0)
            nc.vector.tensor_scalar_mul(w5[:, :], w5[:, :], 512.0)
            nc.vector.tensor_scalar_mul(h5[:, :], h5[:, :], 512.0)
            # corners
            nc.vector.scalar_tensor_tensor(out=out7[:, :, 3], in0=w5[:, :],
                                           scalar=-0.5, in1=cx[:, :],
                                           op0=mybir.AluOpType.mult,
                                           op1=mybir.AluOpType.add)
            nc.vector.scalar_tensor_tensor(out=out7[:, :, 4], in0=h5[:, :],
                                           scalar=-0.5, in1=cy[:, :],
                                           op0=mybir.AluOpType.mult,
                                           op1=mybir.AluOpType.add)
            nc.vector.scalar_tensor_tensor(out=out7[:, :, 5], in0=w5[:, :],
                                           scalar=0.5, in1=cx[:, :],
                                           op0=mybir.AluOpType.mult,
                                           op1=mybir.AluOpType.add)
            nc.vector.scalar_tensor_tensor(out=out7[:, :, 6], in0=h5[:, :],
                                           scalar=0.5, in1=cy[:, :],
                                           op0=mybir.AluOpType.mult,
                                           op1=mybir.AluOpType.add)

            # ---------------- phase 6: write out ----------------
            # out[bb, d, :] with d = 128*half + p lives at out7[p, 2bb+half, :]
            out_ap0 = bass.AP(out[:, :, :].tensor, 0,
                              [[7, 128], [TOPK * 7, BPC], [1, 7]])
            nc.sync.dma_start(out=out_ap0, in_=out7[:, 0:4, :])
            out_ap1 = bass.AP(out[:, :, :].tensor, 128 * 7,
                              [[7, 72], [TOPK * 7, BPC], [1, 7]])
            nc.sync.dma_start(out=out_ap1, in_=out7[0:72, 4:8, :])

    return nc


_cached_nc = None

# test-harness knobs (ignored in normal use)
TRACE = False
LAST_RESULTS = None


def kernel(y_pred: np.ndarray) -> np.ndarray:
    from concourse.bass_utils import run_bass_kernel_spmd

    global _cached_nc, LAST_RESULTS
    if _cached_nc is None:
        _cached_nc = build_nc(debug=False)
    nc = _cached_nc

    y_pred = np.asarray(y_pred, dtype=np.float32)
    conf0 = np.ascontiguousarray(y_pred[0, :, 1:1 + NCONF])
    cst = np.zeros((128, 128 + NCONF), np.float32)
    cst[:, 0:128] = np.eye(128, dtype=np.float32)
    cst[:, 128:] = np.arange(NCONF, dtype=np.float32)[None, :]
    in_maps = []
    for c in range(NCORES):
        shard = np.ascontiguousarray(
            y_pred[c * BPC:(c + 1) * BPC].transpose(1, 0, 2).reshape(N, BPC * ROW))
        in_maps.append({"conf0": conf0, "yp": shard, "cst": cst})

    res = run_bass_kernel_spmd(nc, in_maps, core_ids=list(range(NCORES)),
                               trace=TRACE)
    LAST_RESULTS = res
    out = np.concatenate([res.results[c]["out"] for c in range(NCORES)], axis=0)
    return out


# revision 36
# speedup vs baseline: 1.1655x; 1.0689x over previous
"""DecodeDetections kernel for trn2 (8 NeuronCores, SPMD data-parallel over batch).

Reference semantics (see problem):
  - decode box coords from y_pred[..., 81:93], confidences are cols 1..80
  - top-200 box indices selected from batch item 0's per-box max confidence
  - output [32, 200, 7] = (thresh_met, argmax_class, max_conf, xmin, ymin, xmax, ymax)
    gathered at those 200 indices for every batch item, ordered by descending
    batch-0 max-conf.

Strategy: each core gets 4 batch items (full rows) + a replica of batch-0's
confidence block. On-device: stream batch-0 conf -> per-box class max ->
PE-transpose into box-major [16, 4096] layout -> single gpsimd topk (k=256)
-> indirect-DMA gather of the selected 200 rows for the core's 4 batch items
-> decode only those 800 rows -> [4, 200, 7] out. Host concatenates.
"""

import numpy as np

import concourse.bass as bass
import concourse.bacc as bacc
import concourse.bass_isa as bass_isa
import concourse.mybir as mybir
import concourse.tile as tile
from concourse import library_config


def _gpsimd_topk(nc, out_ap, in_ap, tokens, vocab_size, k):
    # nc.gpsimd.topk minus the isinstance(SBTensorHandle) assert, which
    # rejects Tile-pool symbolic handles.
    eng = nc.gpsimd
    _in = eng.lower_ap(in_ap, for_isa=True)
    _out = eng.lower_ap(out_ap, for_isa=True)
    return eng.add_instruction(
        bass_isa.InstTopk(
            name=f"I-{nc.next_id()}",
            ins=[_in],
            outs=[_out],
            _tokens=tokens,
            _n=vocab_size,
            _k=k,
        )
    )

F32 = mybir.dt.float32
U32 = mybir.dt.uint32
I32 = mybir.dt.int32

N = 24564          # boxes
ROW = 93           # channels per box
NCONF = 80         # class confidences (cols 1..80)
B = 32             # total batch
NCORES = 8
BPC = B // NCORES  # batch items per core
TOPK = 200
K256 = 256
NEG = -1.0e30

CHUNK = 16                      # row-chunks of 128 per conf DMA tile
ROWS_PER_TILE = 128 * CHUNK     # 2048
NFULL = N // ROWS_PER_TILE      # 11 full tiles
TAILROWS = N - NFULL * ROWS_PER_TILE            # 2036
TAILC = TAILROWS // 128                          # 15 full c-chunks
TAILP = TAILROWS - TAILC * 128                   # 116 rows in last chunk
FTOT = 192                       # mc free dim: 24576 / 128
VOCAB = 51200                    # topk padded size (_n is u16; needs >50000, %128)
VPL = VOCAB // 16                # 3200 = 25*128 per partition


def build_nc(debug: bool = False, stage: int = 5):
    nc = _build_raw(debug, stage)
    # insert_library_loads doesn't track the manual load_library(topk) at
    # t=0 and would re-insert a ~28us reload right before the topk
    # instruction; topk is the only library op here, so skip the pass.
    nc.insert_library_loads = lambda: None
    nc.finalize()
    return nc


def _build_raw(debug: bool = False, stage: int = 5):
    nc = bacc.Bacc("TRN2", target_bir_lowering=False, debug=False)

    conf0 = nc.dram_tensor("conf0", [N, NCONF], F32, kind="ExternalInput")
    cst = nc.dram_tensor("cst", [128, 128 + NCONF + K256], F32, kind="ExternalInput")
    yp = nc.dram_tensor("yp", [N, BPC * ROW], F32, kind="ExternalInput")  # box-major
    out = nc.dram_tensor("out", [BPC, TOPK, 7], F32, kind="ExternalOutput")
    idxb = nc.dram_tensor("idxb", [K256], U32)   # bounce: topk indices
    idxb2 = nc.dram_tensor("idxb2", [K256], U32)  # bounce: rank-ordered indices
    dbg = {}
    if debug:
        dbg["mc"] = nc.dram_tensor("dbg_mc", [128, FTOT], F32, kind="ExternalOutput")
        dbg["tko"] = nc.dram_tensor("dbg_tko", [16, 32], U32, kind="ExternalOutput")
        dbg["offs"] = nc.dram_tensor("dbg_offs", [128, 8], U32, kind="ExternalOutput")
        dbg["rank"] = nc.dram_tensor("dbg_rank", [128, 8], F32, kind="ExternalOutput")
        dbg["g"] = nc.dram_tensor("dbg_g", [128, 8, ROW], F32, kind="ExternalOutput")

    with tile.TileContext(nc) as tc:
        with (
            tc.tile_pool(name="conf", bufs=3) as conf_pool,
            tc.tile_pool(name="persist", bufs=1) as persist,
            tc.tile_pool(name="psum", bufs=2, space="PSUM") as psum_pool,
            tc.tile_pool(name="small", bufs=1) as small,
        ):
            # ---------------- persistent tiles ----------------
            mc = persist.tile([128, FTOT], F32)          # per-box class max
            topk_in = persist.tile([16, VPL], F32)       # box-major, padded
            identity = persist.tile([128, 128], F32)
            iota_f = persist.tile([128, NCONF], F32)

            # load the topk gpsimd library immediately: the ~30us ucode IRAM
            # fetch then overlaps the conf scan instead of serializing right
            # before the topk instruction.
            nc.gpsimd.load_library(library_config.topk)
            # constants shipped from host (identity | iota80): using gpsimd
            # iota/affine_select would pin the standard library and delay the
            # ~30us topk-library reload to after the conf scan.
            nc.sync.dma_start(out=identity[:, :], in_=cst[:, 0:128])
            nc.sync.dma_start(out=iota_f[:, :], in_=cst[:, 128:128 + NCONF])
            iota256 = persist.tile([128, K256], F32)
            nc.sync.dma_start(out=iota256[:, :],
                              in_=cst[:, 128 + NCONF:128 + NCONF + K256])
            # partitions 6..15 of topk_in are pure padding; 0..5 are fully
            # overwritten by the reorg DMAs below (engines need an aligned
            # start partition, so memset the whole tile)
            nc.vector.memset(topk_in[:, :], NEG)

            # ---------------- phase 1: conf scan ----------------
            for j in range(NFULL + 1):
                r0 = j * ROWS_PER_TILE
                ct = conf_pool.tile([128, CHUNK, NCONF], F32, tag="ct")
                if j < NFULL:
                    nc.sync.dma_start(
                        out=ct[:, :, :],
                        in_=conf0[r0:r0 + ROWS_PER_TILE, :].rearrange(
                            "(c p) k -> p c k", p=128),
                    )
                else:
                    # engines need aligned start partitions: memset 96..128
                    # first, the tail DMA then overwrites the valid 96..115
                    nc.vector.memset(ct[96:128, TAILC:TAILC + 1, :], NEG)
                    nc.sync.dma_start(
                        out=ct[:, 0:TAILC, :],
                        in_=conf0[r0:r0 + TAILC * 128, :].rearrange(
                            "(c p) k -> p c k", p=128),
                    )
                    nc.sync.dma_start(
                        out=ct[0:TAILP, TAILC:TAILC + 1, :],
                        in_=conf0[r0 + TAILC * 128:N, :].rearrange(
                            "(c p) k -> p c k", p=TAILP),
                    )
                nc.vector.reduce_max(
                    out=mc[:, j * CHUNK:(j + 1) * CHUNK],
                    in_=ct[:, :, :],
                    axis=mybir.AxisListType.X,
                )

            if debug:
                nc.sync.dma_start(out=dbg["mc"][:, :], in_=mc[:, :])
            if stage <= 1:
                return nc

            # ---------------- phase 2: transpose to box-major ----------------
            t1p = psum_pool.tile([128, 128], F32)
            t2p = psum_pool.tile([64, 128], F32)
            nc.tensor.transpose(t1p[:, :], mc[:, 0:128], identity[:, :])
            nc.tensor.transpose(t2p[:, :], mc[:, 128:192], identity[:, :])
            t1s = small.tile([128, 128], F32)
            t2s = small.tile([64, 128], F32)
            nc.vector.tensor_copy(t1s[:, :], t1p[:, :])
            nc.vector.tensor_copy(t2s[:, :], t2p[:, :])
            # t1s[f, p] = box(128f + p), f in [0,128)
            # t2s[f, p] = box(16384 + 128f + p), f in [0,64)
            # topk_in partition P covers boxes [3200P, 3200P+3200); the
            # T1/T2 seam (box 16384) is mid-partition-5, hence 5 DMAs.
            nc.sync.dma_start(
                out=topk_in[0:5, :].rearrange("P (a p) -> P a p", p=128),
                in_=t1s[0:125, :])
            nc.sync.dma_start(
                out=topk_in[5:6, 0:384].rearrange("P (a p) -> P a p", p=128),
                in_=t1s[125:128, :])
            nc.sync.dma_start(
                out=topk_in[5:6, 384:3200].rearrange("P (a p) -> P a p", p=128),
                in_=t2s[0:22, :])
            nc.sync.dma_start(
                out=topk_in[6:7, :].rearrange("P (a p) -> P a p", p=128),
                in_=t2s[22:47, :])
            nc.sync.dma_start(
                out=topk_in[7:8, 0:2176].rearrange("P (a p) -> P a p", p=128),
                in_=t2s[47:64, :])

            # ---------------- phase 3: topk ----------------
            tko = small.tile([16, 32], U32)
            _gpsimd_topk(nc, tko[:, :], topk_in[:, :], tokens=1,
                         vocab_size=VOCAB, k=K256)
            if debug:
                nc.sync.dma_start(out=dbg["tko"][:, :], in_=tko[:, :])
            if stage <= 2:
                return nc

            # bounce indices (topk's own order, ties arbitrary) to DRAM
            nc.sync.dma_start(out=idxb[:], in_=tko[:, 16:32])

            # ---- exact re-rank of the 256 candidates ----
            # The input has many exact-tie confidence values; jax.lax.top_k
            # orders ties by ascending box index, and the topk ucode's tie
            # order is unspecified. Recompute each candidate's exact rank:
            #   rank_c = #{j: v_j > v_c} + #{j: v_j == v_c and idx_j < idx_c}
            # candidate slot s = p + 128h  ->  (partition p, half h)
            # NOTE hw indirect DMA: ONE offset per partition, and keep all
            # indirect-call APs as clean contiguous 2-D tiles.
            icol = [small.tile([128, 1], U32, tag=f"icol{h}", name=f"icol{h}")
                    for h in range(2)]
            for h in range(2):
                nc.sync.dma_start(
                    out=icol[h][:, :],
                    in_=bass.AP(idxb[:].tensor, 128 * h, [[1, 128], [1, 1]]))
            # candidate values come straight from the topk output's value
            # half (same f32 bits as mc) -- bounce to DRAM, read back as a
            # per-partition column and a 128-way broadcast row, bitcast u32->f32
            vb0 = nc.dram_tensor("vb0", [K256], U32)
            nc.sync.dma_start(out=vb0[:], in_=tko[:, 0:16])
            vcol_u = small.tile([128, 2], U32)
            nc.sync.dma_start(
                out=vcol_u[:, :],
                in_=bass.AP(vb0[:].tensor, 0, [[1, 128], [128, 2]]))
            vcol = vcol_u[:, :].bitcast(F32)
            idxf = small.tile([128, 2], F32)
            for h in range(2):
                nc.vector.tensor_copy(idxf[:, h:h + 1], icol[h][:, :])  # ->f32
            vrow_u = small.tile([128, K256], U32)
            nc.sync.dma_start(
                out=vrow_u[:, :],
                in_=bass.AP(vb0[:].tensor, 0, [[0, 128], [1, K256]]))
            vrow = vrow_u[:, :].bitcast(F32)
            irow_u = small.tile([128, K256], U32)
            nc.sync.dma_start(
                out=irow_u[:, :],
                in_=bass.AP(idxb[:].tensor, 0, [[0, 128], [1, K256]]))
            irow_ps = small.tile([128, K256], F32)
            nc.vector.tensor_copy(irow_ps[:, :], irow_u[:, :])

            frank = small.tile([128, 2], F32)
            for h in range(2):
                j1 = small.tile([128, K256], F32, tag=f"j1_{h}")
                eqm = small.tile([128, K256], F32, tag=f"eq_{h}")
                j2 = small.tile([128, K256], F32, tag=f"j2_{h}")
                r1 = small.tile([128, 1], F32, tag=f"r1_{h}")
                r2 = small.tile([128, 1], F32, tag=f"r2_{h}")
                nc.vector.tensor_scalar(
                    out=j1[:, :], in0=vrow, scalar1=vcol[:, h:h + 1],
                    scalar2=None, op0=mybir.AluOpType.is_gt,
                    op1=mybir.AluOpType.add, accum_out=r1[:, :])
                nc.vector.tensor_scalar(
                    out=eqm[:, :], in0=vrow, scalar1=vcol[:, h:h + 1],
                    scalar2=None, op0=mybir.AluOpType.is_equal)
                nc.vector.scalar_tensor_tensor(
                    out=j2[:, :], in0=irow_ps[:, :], scalar=idxf[:, h:h + 1],
                    in1=eqm[:, :], op0=mybir.AluOpType.is_lt,
                    op1=mybir.AluOpType.mult)
                nc.vector.reduce_sum(out=r2[:, :], in_=j2[:, :],
                                     axis=mybir.AxisListType.X)
                nc.vector.tensor_tensor(out=frank[:, h:h + 1], in0=r1[:, :],
                                        in1=r2[:, :], op=mybir.AluOpType.add)

            # permute candidate indices into rank order with a one-hot
            # matmul (exact: one-hot entries are 0/1, indices < 2^24), then
            # one contiguous 1KB DRAM bounce. (An indirect scatter of 256
            # single words costs ~9us per call in scattered DRAM writes.)
            oh = [small.tile([128, K256], F32, tag=f"oh{h}", name=f"oh{h}")
                  for h in range(2)]
            sidx_ps = psum_pool.tile([1, K256], F32)
            for h in range(2):
                nc.vector.tensor_scalar(
                    out=oh[h][:, :], in0=iota256[:, :], scalar1=frank[:, h:h + 1],
                    scalar2=None, op0=mybir.AluOpType.is_equal)
                nc.tensor.matmul(sidx_ps[:, :], lhsT=idxf[:, h:h + 1],
                                 rhs=oh[h][:, :], start=(h == 0), stop=(h == 1))
            sidx = small.tile([1, K256], F32)
            sidx_u = small.tile([1, K256], U32)
            nc.vector.tensor_copy(sidx[:, :], sidx_ps[:, :])
            nc.vector.tensor_copy(sidx_u[:, :], sidx[:, :])  # f32 -> u32
            nc.sync.dma_start(out=idxb2[:], in_=sidx_u[:, :])

            # bo[h][p] = box index with final rank d = 128*half + p
            bo = [small.tile([128, 1], U32, tag=f"bo{h}", name=f"bo{h}")
                  for h in range(2)]
            for h in range(2):
                nc.sync.dma_start(
                    out=bo[h][:, :],
                    in_=bass.AP(idxb2[:].tensor, 128 * h, [[1, 128], [1, 1]]))
            if debug:
                offs_mega = small.tile([128, 8], U32)
                nc.vector.memset(offs_mega[:, :], 0)
                for h in range(2):
                    nc.vector.tensor_copy(offs_mega[:, h:h + 1], bo[h][:, :])
                nc.sync.dma_start(out=dbg["offs"][:, :], in_=offs_mega[:, :])
            if stage <= 3:
                return nc

            # ---------------- phase 4: gather ----------------
            # yp is box-major [N, 4*93]: one index fetches all 4 batch rows.
            # g column i = 4*half + bb.
            g = persist.tile([128, 8, ROW], F32)
            for h in range(2):
                gh = small.tile([128, BPC * ROW], F32, tag=f"gh{h}", name=f"gh{h}")
                nc.gpsimd.indirect_dma_start(
                    out=gh[:, :],
                    out_offset=None,
                    in_=yp[:, :],
                    in_offset=bass.IndirectOffsetOnAxis(ap=bo[h][:, :], axis=0),
                )
                nc.vector.tensor_copy(g[:, 4 * h:4 * h + 4, :],
                                      gh[:, :].rearrange("p (b r) -> p b r", r=ROW))
            if debug:
                nc.sync.dma_start(out=dbg["g"][:, :, :], in_=g[:, :, :])
            if stage <= 4:
                return nc

            # ---------------- phase 5: decode ----------------
            out7 = persist.tile([128, 8, 7], F32)
            conf = g[:, :, 1:1 + NCONF]                    # [128, 8, 80]
            mxc = small.tile([128, 8], F32)
            nc.vector.reduce_max(out=mxc[:, :], in_=conf, axis=mybir.AxisListType.X)

            # argmax via (iota - 256*eq) reduce_min
            eq = small.tile([128, 8, NCONF], F32)
            mxc_b = bass.AP(mxc[:, :].tensor, mxc[:, :].offset,
                            [list(mxc[:, :].ap[0]), list(mxc[:, :].ap[1]), [0, NCONF]])
            nc.vector.tensor_tensor(out=eq[:, :, :], in0=conf, in1=mxc_b,
                                    op=mybir.AluOpType.is_equal)
            iota_b = bass.AP(iota_f[:, :].tensor, iota_f[:, :].offset,
                             [list(iota_f[:, :].ap[0]), [0, 8], [1, NCONF]])
            cand = small.tile([128, 8, NCONF], F32)
            nc.vector.scalar_tensor_tensor(
                out=cand[:, :, :], in0=eq[:, :, :], scalar=-256.0, in1=iota_b,
                op0=mybir.AluOpType.mult, op1=mybir.AluOpType.add)
            amx = small.tile([128, 8], F32)
            nc.vector.tensor_reduce(out=amx[:, :], in_=cand[:, :, :],
                                    axis=mybir.AxisListType.X,
                                    op=mybir.AluOpType.min)
            nc.vector.tensor_scalar(out=out7[:, :, 1], in0=amx[:, :], scalar1=256.0,
                                    scalar2=None, op0=mybir.AluOpType.add)
            nc.vector.tensor_scalar(out=out7[:, :, 0], in0=mxc[:, :], scalar1=0.5,
                                    scalar2=None, op0=mybir.AluOpType.is_gt)
            nc.vector.tensor_copy(out7[:, :, 2], mxc[:, :])

            def col(k):
                return g[:, :, 81 + k]

            tmp = small.tile([128, 8], F32)
            cx = small.tile([128, 8], F32)
            cy = small.tile([128, 8], F32)
            w5 = small.tile([128, 8], F32)
            h5 = small.tile([128, 8], F32)

            # cx = ((c0*c8)*c6 + c4) ; cy = ((c1*c9)*c7 + c5)
            nc.vector.tensor_tensor(out=tmp[:, :], in0=col(0), in1=col(8),
                                    op=mybir.AluOpType.mult)
            nc.vector.tensor_tensor(out=tmp[:, :], in0=tmp[:, :], in1=col(6),
                                    op=mybir.AluOpType.mult)
            nc.vector.tensor_tensor(out=cx[:, :], in0=tmp[:, :], in1=col(4),
                                    op=mybir.AluOpType.add)
            nc.vector.tensor_tensor(out=tmp[:, :], in0=col(1), in1=col(9),
                                    op=mybir.AluOpType.mult)
            nc.vector.tensor_tensor(out=tmp[:, :], in0=tmp[:, :], in1=col(7),
                                    op=mybir.AluOpType.mult)
            nc.vector.tensor_tensor(out=cy[:, :], in0=tmp[:, :], in1=col(5),
                                    op=mybir.AluOpType.add)
            # w = exp(c2*c10)*c6 ; h = exp(c3*c11)*c7   (then * 512)
            # Precise f32 exp on DVE (ACT's Exp LUT is only ~2e-4 accurate):
            # k = round(x/ln2) via the magic-constant trick, 3-term
            # Cody-Waite reduction, degree-7 Taylor Horner, exact 2^k by
            # integer-constructing the f32 bit pattern and bitcasting.
            INV_LN2 = 1.4426950408889634
            MAGIC = 12582912.0          # 1.5 * 2^23: round-to-nearest
            CW1, CW2, CW3 = 0.693359375, -2.1219444e-4, 1.6465718e-12
            FACT = [1.0, 1.0, 0.5, 1.0 / 6, 1.0 / 24, 1.0 / 120, 1.0 / 720,
                    1.0 / 5040]
            xe = small.tile([128, 16], F32)
            nc.vector.tensor_tensor(out=xe[:, 0:8], in0=col(2), in1=col(10),
                                    op=mybir.AluOpType.mult)
            nc.vector.tensor_tensor(out=xe[:, 8:16], in0=col(3), in1=col(11),
                                    op=mybir.AluOpType.mult)
            kf = small.tile([128, 16], F32)
            nc.vector.tensor_scalar(out=kf[:, :], in0=xe[:, :], scalar1=INV_LN2,
                                    scalar2=None, op0=mybir.AluOpType.mult)
            nc.vector.tensor_scalar(out=kf[:, :], in0=kf[:, :], scalar1=MAGIC,
                                    scalar2=MAGIC, op0=mybir.AluOpType.add,
                                    op1=mybir.AluOpType.subtract)
            rr = small.tile([128, 16], F32)
            nc.vector.scalar_tensor_tensor(
                out=rr[:, :], in0=kf[:, :], scalar=-CW1, in1=xe[:, :],
                op0=mybir.AluOpType.mult, op1=mybir.AluOpType.add)
            nc.vector.scalar_tensor_tensor(
                out=rr[:, :], in0=kf[:, :], scalar=-CW2, in1=rr[:, :],
                op0=mybir.AluOpType.mult, op1=mybir.AluOpType.add)
            nc.vector.scalar_tensor_tensor(
                out=rr[:, :], in0=kf[:, :], scalar=-CW3, in1=rr[:, :],
                op0=mybir.AluOpType.mult, op1=mybir.AluOpType.add)
            pp = small.tile([128, 16], F32)
            pq = small.tile([128, 16], F32)
            nc.vector.memset(pp[:, :], FACT[7])
            for kdeg in range(6, -1, -1):
                nc.vector.tensor_tensor(out=pq[:, :], in0=pp[:, :], in1=rr[:, :],
                                        op=mybir.AluOpType.mult)
                nc.vector.tensor_scalar(out=pp[:, :], in0=pq[:, :],
                                        scalar1=FACT[kdeg], scalar2=None,
                                        op0=mybir.AluOpType.add)
            # 2^k: bits = (k+127) * 2^23, exact in f32; value-cast to u32
            # and bitcast back to f32
            bitsf = small.tile([128, 16], F32)
            nc.vector.tensor_scalar(out=bitsf[:, :], in0=kf[:, :], scalar1=127.0,
                                    scalar2=8388608.0, op0=mybir.AluOpType.add,
                                    op1=mybir.AluOpType.mult)
            bitsu = small.tile([128, 16], U32)
            nc.vector.tensor_copy(bitsu[:, :], bitsf[:, :])
            exv = small.tile([128, 16], F32)
            nc.vector.tensor_tensor(out=exv[:, :], in0=pp[:, :],
                                    in1=bitsu[:, :].bitcast(F32),
                                    op=mybir.AluOpType.mult)
            nc.vector.tensor_tensor(out=w5[:, :], in0=exv[:, 0:8], in1=col(6),
                                    op=mybir.AluOpType.mult)
            nc.vector.tensor_tensor(out=h5[:, :], in0=exv[:, 8:16], in1=col(7),
                                    op=mybir.AluOpType.mult)
            # scale by 512 (exact)
            nc.vector.tensor_scalar_mul(cx[:, :], cx[:, :], 512.0)
            nc.vector.tensor_scalar_mul(cy[:, :], cy[:, :], 512.0)
            nc.vector.tensor_scalar_mul(w5[:, :], w5[:, :], 512.0)
            nc.vector.tensor_scalar_mul(h5[:, :], h5[:, :], 512.0)
            # corners
            nc.vector.scalar_tensor_tensor(out=out7[:, :, 3], in0=w5[:, :],
                                           scalar=-0.5, in1=cx[:, :],
                                           op0=mybir.AluOpType.mult,
                                           op1=mybir.AluOpType.add)
            nc.vector.scalar_tensor_tensor(out=out7[:, :, 4], in0=h5[:, :],
                                           scalar=-0.5, in1=cy[:, :],
                                           op0=mybir.AluOpType.mult,
                                           op1=mybir.AluOpType.add)
            nc.vector.scalar_tensor_tensor(out=out7[:, :, 5], in0=w5[:, :],
                                           scalar=0.5, in1=cx[:, :],
                                           op0=mybir.AluOpType.mult,
                                           op1=mybir.AluOpType.add)
            nc.vector.scalar_tensor_tensor(out=out7[:, :, 6], in0=h5[:, :],
                                           scalar=0.5, in1=cy[:, :],
                                           op0=mybir.AluOpType.mult,
                                           op1=mybir.AluOpType.add)

            # ---------------- phase 6: write out ----------------
            # out[bb, d, :] with d = 128*half + p lives at out7[p, 2bb+half, :]
            out_ap0 = bass.AP(out[:, :, :].tensor, 0,
                              [[7, 128], [TOPK * 7, BPC], [1, 7]])
            nc.sync.dma_start(out=out_ap0, in_=out7[:, 0:4, :])
            out_ap1 = bass.AP(out[:, :, :].tensor, 128 * 7,
                              [[7, 72], [TOPK * 7, BPC], [1, 7]])
            nc.sync.dma_start(out=out_ap1, in_=out7[0:72, 4:8, :])

    return nc


_cached_nc = None

# test-harness knobs (ignored in normal use)
TRACE = False
LAST_RESULTS = None


def kernel(y_pred: np.ndarray) -> np.ndarray:
    from concourse.bass_utils import run_bass_kernel_spmd

    global _cached_nc, LAST_RESULTS
    if _cached_nc is None:
        _cached_nc = build_nc(debug=False)
    nc = _cached_nc

    y_pred = np.asarray(y_pred, dtype=np.float32)
    conf0 = np.ascontiguousarray(y_pred[0, :, 1:1 + NCONF])
    cst = np.zeros((128, 128 + NCONF + K256), np.float32)
    cst[:, 0:128] = np.eye(128, dtype=np.float32)
    cst[:, 128:128 + NCONF] = np.arange(NCONF, dtype=np.float32)[None, :]
    cst[:, 128 + NCONF:] = np.arange(K256, dtype=np.float32)[None, :]
    in_maps = []
    for c in range(NCORES):
        shard = np.ascontiguousarray(
            y_pred[c * BPC:(c + 1) * BPC].transpose(1, 0, 2).reshape(N, BPC * ROW))
        in_maps.append({"conf0": conf0, "yp": shard, "cst": cst})

    res = run_bass_kernel_spmd(nc, in_maps, core_ids=list(range(NCORES)),
                               trace=TRACE)
    LAST_RESULTS = res
    out = np.concatenate([res.results[c]["out"] for c in range(NCORES)], axis=0)
    return out
